# revision 27
# baseline (speedup 1.0000x reference)
"""NT-Xent contrastive loss on 8 Trainium2 NeuronCores (Bass/Tile), v10.

Slab-cover strategy (no collectives): core c loads fp8 slabs
S_c = {c, c+1, c+2, c+4} (mod 8); every slab pair meets on some core, so
each of the 36 unique 1024x1024 sim blocks is computed once globally.

Per-core kernel (vs the v1 baseline, 2x faster):
  * fp8(e4m3) inputs, host pre-laid-out to the exact SBUF tile layout
    [128, slab, kchunk, row] -> contiguous 16 KiB DMA descriptors
    (8 MiB/core vs 32 MiB); DMA issue order = HBM priority (W, slab-0
    chunks, then slabs 1-3).
  * All heavy matmuls run fp8 DoubleRow (K=256/instruction): head
    projection (W*32 fp8; bias added during PSUM evacuation via
    tensor_scalar), sim blocks, and exp column-sums over mb-paired
    fp8e5 exp tiles (ones[128,2,1] x exp[128,2,512]).
  * L2-normalize: normsq via ones-matmul on bf16 squares; rsqrt as
    Exp(-0.5*Ln(x)) with walrus steered (via get_activation_tables) to
    the single natural_log_exp_and_others table set -> no table thrash;
    1/norm broadcast by K=1 ones-row matmul; normalize-mult emits fp8.
  * Software pipeline: each block's 4 sim pairs interleave with the
    next slab's head chains (PE never FIFO-parks on the ACT exp
    backlog); colsum chains split so their last matmul never blocks;
    rsqrt chain emitted mid-phase.
  * Symmetric blocks (B0 diag, B4 = pair {c, c+4}, whose transpose is
    computed by core c+4) use a STAIRCASE: only cols >= 256*pair; host
    reassembles full sums as rowsum_P + colsum_P - diag-square overlap
    (dsq output). Cuts 6 of 40 exp-tile-equivalents of ScalarE work.
  * Diagonal self-sim exp(10*|u8_i|^2) recomputed exactly from the fp8
    vectors (DVE square + ones-matmul + ACT exp) and subtracted on host.
  * pos term: log(pos) = 10*possim computed without exp.
  * PSUM: head 2 banks, sim 4 (double-buffered), normsq 1, colsum 1.
"""
import numpy as np
import ml_dtypes

SLOTS = [(c, (c + 1) % 8, (c + 2) % 8, (c + 4) % 8) for c in range(8)]
WSCALE = 32.0  # power of two; normalize() cancels it exactly

_CACHE = {}


def _build():
    if "nc" in _CACHE:
        return _CACHE["nc"]
    import concourse.bacc as bacc
    import concourse.tile as tile
    import concourse.mybir as mybir

    F32 = mybir.dt.float32
    BF16 = mybir.dt.bfloat16
    F8E4 = mybir.dt.float8e4
    F8E5 = mybir.dt.float8e5
    AF = mybir.ActivationFunctionType
    ALU = mybir.AluOpType
    DR = mybir.MatmulPerfMode.DoubleRow

    # Steer walrus act-table selection: keep Exp/Ln only in the combined
    # natural_log_exp_and_others set so the kernel needs ONE table load
    # instead of thrashing exp_and_others <-> natural_log (1.28us each).
    _orig_gat = bacc.get_activation_tables

    def _gat(arch):
        t = _orig_gat(arch)
        for name, fns in t.items():
            if name != "natural_log_exp_and_others":
                fns.discard(mybir.ActivationFunctionType.Exp)
                fns.discard(mybir.ActivationFunctionType.Ln)
        return t

    bacc.get_activation_tables = _gat

    nc = bacc.Bacc("TRN2", num_devices=8, debug=False)
    a_emb = nc.dram_tensor("emb8", [128, 4, 16, 1024], F8E4,
                           kind="ExternalInput").ap()
    a_W = nc.dram_tensor("W8", [128, 2, 16, 128], F8E4, kind="ExternalInput").ap()
    a_b = nc.dram_tensor("bS", [128, 2], F32, kind="ExternalInput").ap()
    a_oc = nc.dram_tensor("ones_col", [128, 1], BF16, kind="ExternalInput").ap()
    a_or = nc.dram_tensor("ones_row", [1, 512], BF16, kind="ExternalInput").ap()
    a_o8 = nc.dram_tensor("ones8", [128, 32], F8E5, kind="ExternalInput").ap()
    o_rp = nc.dram_tensor("rowpart", [128, 40], F32, kind="ExternalOutput").ap()
    o_cp = nc.dram_tensor("colpart", [5, 1024], F32, kind="ExternalOutput").ap()
    o_dg = nc.dram_tensor("diagexp", [1, 1024], F32, kind="ExternalOutput").ap()
    o_dq = nc.dram_tensor("dsq", [2, 1024], F32, kind="ExternalOutput").ap()
    o_ps = nc.dram_tensor("possim", [1, 1024], F32, kind="ExternalOutput").ap()

    with tile.TileContext(nc) as tc:
        with tc.tile_pool(name="sb", bufs=1) as sb, \
             tc.tile_pool(name="emb", bufs=4) as embp, \
             tc.tile_pool(name="hp", bufs=2) as hp, \
             tc.tile_pool(name="sq", bufs=2) as sqp, \
             tc.tile_pool(name="rn", bufs=2) as rnp, \
             tc.tile_pool(name="ln", bufs=2) as lnp, \
             tc.tile_pool(name="expp", bufs=9) as expp, \
             tc.tile_pool(name="headp", bufs=2, space="PSUM") as headp, \
             tc.tile_pool(name="simp", bufs=2, space="PSUM") as simp, \
             tc.tile_pool(name="nsp", bufs=1, space="PSUM") as nsp, \
             tc.tile_pool(name="csp", bufs=1, space="PSUM") as csp:

            t_oc = sb.tile([128, 1], BF16, name="t_oc")
            nc.gpsimd.dma_start(t_oc[:], a_oc[:])
            t_or = sb.tile([1, 512], BF16, name="t_or")
            nc.gpsimd.dma_start(t_or[:], a_or[:])
            t_o8 = sb.tile([128, 2, 16], F8E5, name="t_o8")
            nc.gpsimd.dma_start(t_o8[:], a_o8.rearrange("p (a o) -> p a o", o=16))
            t_b = sb.tile([128, 2], F32, name="t_b")
            nc.gpsimd.dma_start(t_b[:], a_b[:])
            t_W = sb.tile([128, 2, 16, 128], F8E4, name="t_W")
            for dh in range(2):
                nc.sync.dma_start(t_W[:, dh, :, :], a_W[:, dh, :, :])
            # slab 0 in four chunk tiles so head chains start on chunk 0
            t_e0c = []
            for ch in range(4):
                tec = embp.tile([128, 4, 1024], F8E4, name=f"t_e0c{ch}",
                                tag="emb")
                nc.sync.dma_start(tec[:], a_emb[:, 0, 4 * ch:4 * ch + 4, :])
                t_e0c.append(tec)
            t_e = [t_e0c]
            for k in range(1, 4):
                te = embp.tile([128, 16, 1024], F8E4, name=f"t_e{k}", tag="emb")
                nc.sync.dma_start(te[:], a_emb[:, k, :, :])
                t_e.append(te)

            # persistent normalized slabs (fp8) and staging accumulators
            t_on = [sb.tile([128, 2, 1024], F8E4, name=f"t_on{k}")
                    for k in range(4)]
            rp_st = sb.tile([128, 5, 8], F32, name="rp_st")
            cp_st = sb.tile([1, 5120], F32, name="cp_st")
            dq_st = sb.tile([1, 2048], F32, name="dq_st")
            dg_st = sb.tile([1, 1024], F32, name="dg_st")
            ps_st = sb.tile([1, 1024], F32, name="ps_st")

            def head_chain(k, th, tsq, h, dh):
                """one (h, dh) quarter of slab k's head projection."""
                ph = headp.tile([128, 512], F32, name="p_h", tag="head")
                for j in range(8):
                    if isinstance(t_e[k], list):
                        rhs = t_e[k][j // 2][:, 2 * (j % 2):2 * (j % 2) + 2,
                                             h * 512:(h + 1) * 512]
                    else:
                        rhs = t_e[k][:, 2 * j:2 * j + 2,
                                     h * 512:(h + 1) * 512]
                    nc.tensor.matmul(
                        ph[:],
                        t_W[:, dh, 2 * j:2 * j + 2, :],
                        rhs,
                        start=(j == 0), stop=(j == 7), perf_mode=DR)
                # evacuate + bias add (per-partition scalar b[d]) in one op
                nc.vector.tensor_scalar_add(
                    th[:, dh, h * 512:(h + 1) * 512], ph[:],
                    t_b[:, dh:dh + 1])
                nc.vector.tensor_tensor(
                    tsq[:, dh, h * 512:(h + 1) * 512],
                    th[:, dh, h * 512:(h + 1) * 512],
                    th[:, dh, h * 512:(h + 1) * 512], ALU.mult)

            def norm_half(tsq, rn, h):
                """normsq + rsqrt (Ln,Exp) for rows h*512..h*512+511."""
                pns = nsp.tile([1, 512], F32, name="p_ns", tag="ns")
                for dh in range(2):
                    nc.tensor.matmul(
                        pns[:], t_oc[:], tsq[:, dh, h * 512:(h + 1) * 512],
                        start=(dh == 0), stop=(dh == 1))
                tln = lnp.tile([1, 512], F32, name="t_ln", tag="ln")
                nc.scalar.activation(tln[:], pns[:], AF.Ln)
                nc.scalar.activation(rn[0:1, h * 512:(h + 1) * 512],
                                     tln[:], AF.Exp, scale=-0.5)

            def stage_finish(k, th, rn, h):
                """broadcast 1/norm, emit fp8 normalized half-slab h."""
                pbc = headp.tile([128, 512], F32, name="p_bc", tag="head")
                nc.tensor.matmul(pbc[:], t_or[0:1, 0:128],
                                 rn[0:1, h * 512:(h + 1) * 512],
                                 start=True, stop=True)
                for dh in range(2):
                    nc.vector.tensor_tensor(
                        t_on[k][:, dh, h * 512:(h + 1) * 512],
                        th[:, dh, h * 512:(h + 1) * 512],
                        pbc[:], ALU.mult)

            def sim_pair(bslot, a, bm, pair, stair=False):
                """two mb tiles of a sim block -> one fp8e5 exp pair tile.
                stair: compute only cols >= 256*pair (symmetric blocks whose
                transpose is covered elsewhere; host reassembles)."""
                lo = 256 * pair if stair else 0
                texp = expp.tile([128, 2, 1024], F8E5, name="t_exp", tag="exp")
                for half in range(2):
                    mb = 2 * pair + half
                    psim = simp.tile([128, 1024], F32, name="p_sim", tag="sim")
                    for nb in range(2):
                        nlo = max(nb * 512, lo)
                        if nlo < (nb + 1) * 512:
                            nc.tensor.matmul(
                                psim[:, nlo:(nb + 1) * 512],
                                t_on[a][:, :, mb * 128:(mb + 1) * 128],
                                t_on[bm][:, :, nlo:(nb + 1) * 512],
                                start=True, stop=True, perf_mode=DR)
                    nc.scalar.activation(
                        texp[:, half, lo:], psim[:, lo:], AF.Exp, scale=10.0,
                        accum_out=rp_st[:, bslot, mb:mb + 1])
                return texp

            def block_cs_start(bslot, texps, stair=False):
                """column-sum chains over all but the last contributing exp
                pair (whose exps may still be in flight on ACT)."""
                pcss = []
                for nb in range(2):
                    pairs = [p for p in range(4)
                             if not stair or 256 * p < (nb + 1) * 512]
                    pcs = csp.tile([1, 512], F32, name="p_cs", tag="cs")
                    pcss.append((pcs, pairs))
                    for pair in pairs[:-1]:
                        nlo = max(nb * 512, 256 * pair if stair else 0)
                        nc.tensor.matmul(
                            pcs[:, nlo - nb * 512:512], t_o8[:, :, 0:1],
                            texps[pair][:, :, nlo:(nb + 1) * 512],
                            start=(pair == pairs[0]), stop=False,
                            perf_mode=DR)
                return pcss

            def block_cs_finish(bslot, texps, pcss, stair=False):
                for nb in range(2):
                    pcs, pairs = pcss[nb]
                    pair = pairs[-1]
                    nlo = max(nb * 512, 256 * pair if stair else 0)
                    nc.tensor.matmul(
                        pcs[:, nlo - nb * 512:512], t_o8[:, :, 0:1],
                        texps[pair][:, :, nlo:(nb + 1) * 512],
                        start=False, stop=True, perf_mode=DR)
                    nc.vector.tensor_copy(
                        cp_st[0:1, 1024 * bslot + nb * 512:
                              1024 * bslot + (nb + 1) * 512],
                        pcs[:])
                # ship this block's outputs now (tiny DMAs, idle gpsimd queue)
                nc.gpsimd.dma_start(
                    o_cp[bslot:bslot + 1, :],
                    cp_st[0:1, 1024 * bslot:1024 * (bslot + 1)])
                nc.gpsimd.dma_start(o_rp[:, 8 * bslot:8 * (bslot + 1)],
                                    rp_st[:, bslot, :])

            def block_dsq_one(slot, texps, pair):
                pdq = csp.tile([1, 256], F32, name="p_dq", tag="cs")
                nc.tensor.matmul(
                    pdq[:], t_o8[:, :, 0:1],
                    texps[pair][:, :, 256 * pair:256 * (pair + 1)],
                    start=True, stop=True, perf_mode=DR)
                nc.vector.tensor_copy(
                    dq_st[0:1, 1024 * slot + 256 * pair:
                          1024 * slot + 256 * (pair + 1)], pdq[:])

            def block_dsq(slot, texps):
                """column sums of the four diagonal 256x256 squares of a
                staircase block -> dq_st[slot*1024 : (slot+1)*1024]."""
                for pair in range(4):
                    block_dsq_one(slot, texps, pair)



            def phase(bslot, a, bm, nxt=None, cs_prev=None, stair=False,
                      self_dsq=None):
                """block (bslot): 4 sim pairs, each followed by one head
                chain of the NEXT slab's stage, so the strict-FIFO PE queue
                always has head work while ACT drains the exp backlog and
                frees sim-psum buffers. Stage k+1's rsqrt chain is emitted
                mid-phase so its ACT ops sit ahead of half the exps."""
                th = tsq = rn = None
                pcss = None
                if cs_prev is not None:
                    pcss = block_cs_start(*cs_prev)
                if nxt is not None:
                    th = hp.tile([128, 2, 1024], BF16, name="t_h", tag="th")
                    tsq = sqp.tile([128, 2, 1024], BF16, name="t_sq", tag="sq")
                    rn = rnp.tile([1, 1024], BF16, name="t_rn", tag="rn")
                texps = []
                for pair in range(4):
                    texps.append(sim_pair(bslot, a, bm, pair, stair=stair))
                    if pair == 0 and pcss is not None:
                        block_cs_finish(cs_prev[0], cs_prev[1], pcss,
                                        stair=cs_prev[2])
                    if pair == 1 and pcss is not None and cs_prev[2]:
                        block_dsq(0 if cs_prev[0] == 0 else 1, cs_prev[1])
                    if self_dsq is not None and pair >= 1:
                        block_dsq_one(self_dsq, texps, pair - 1)
                    if nxt is not None:
                        head_chain(nxt, th, tsq, h=pair // 2, dh=pair % 2)
                        if pair == 1:
                            norm_half(tsq, rn, 0)
                        elif pair == 2:
                            stage_finish(nxt, th, rn, 0)
                        elif pair == 3:
                            norm_half(tsq, rn, 1)
                if nxt is not None:
                    stage_finish(nxt, th, rn, 1)
                return texps

            def colreduce_exp(src8, dst, scale):
                """dst[1,1024] = f(sum_d src8a[d,:]*src8b[d,:])."""
                tq = sqp.tile([128, 2, 1024], BF16, name="t_q", tag="sq")
                nc.vector.tensor_tensor(tq[:], src8[0][:], src8[1][:],
                                        ALU.mult)
                for nb in range(2):
                    pr = nsp.tile([1, 512], F32, name="p_r", tag="ns")
                    for dh in range(2):
                        nc.tensor.matmul(
                            pr[:], t_oc[:], tq[:, dh, nb * 512:(nb + 1) * 512],
                            start=(dh == 0), stop=(dh == 1))
                    if scale is None:
                        nc.vector.tensor_copy(
                            dst[0:1, nb * 512:(nb + 1) * 512], pr[:])
                    else:
                        nc.scalar.activation(
                            dst[0:1, nb * 512:(nb + 1) * 512], pr[:],
                            AF.Exp, scale=scale)

            # slab 0 head alone, then software-pipelined phases
            th0 = hp.tile([128, 2, 1024], BF16, name="t_h", tag="th")
            tsq0 = sqp.tile([128, 2, 1024], BF16, name="t_sq", tag="sq")
            rn0 = rnp.tile([1, 1024], BF16, name="t_rn", tag="rn")
            for h in range(2):
                for dh in range(2):
                    head_chain(0, th0, tsq0, h, dh)
                norm_half(tsq0, rn0, h)
                stage_finish(0, th0, rn0, h)

            tx0 = phase(0, 0, 0, nxt=1, stair=True)
            # diag exp values: exp(10 * |u8_i|^2) == exp(10 * sim_ii)
            colreduce_exp((t_on[0], t_on[0]), dg_st, 10.0)
            tx1 = phase(1, 0, 1, nxt=2, cs_prev=(0, tx0, True))
            tx2 = phase(2, 0, 2, nxt=3, cs_prev=(1, tx1, False))
            # pos: possim_i = sum_d u0[d,i]*u3[d,i]; host uses 10*possim
            colreduce_exp((t_on[0], t_on[3]), ps_st, None)
            tx3 = phase(3, 1, 3, cs_prev=(2, tx2, False))
            tx4 = phase(4, 0, 3, cs_prev=(3, tx3, False), stair=True,
                        self_dsq=1)
            pc4 = block_cs_start(4, tx4, stair=True)
            block_cs_finish(4, tx4, pc4, stair=True)
            block_dsq_one(1, tx4, 3)


            nc.gpsimd.dma_start(o_dg[:], dg_st[:])
            nc.gpsimd.dma_start(o_dq.rearrange("a r -> (a r)")[None, :],
                                dq_st[:])
            nc.gpsimd.dma_start(o_ps[:], ps_st[:])

    try:
        nc.compile()
    finally:
        bacc.get_activation_tables = _orig_gat
    _CACHE["nc"] = nc
    return nc


def _host_inputs(embedded_data, W, b):
    emb = np.asarray(embedded_data, dtype=np.float32)      # [8192, 2048]
    W = np.asarray(W, dtype=np.float32)
    b = np.asarray(b, dtype=np.float32)
    # slab s tile layout: [128(p), 16(kc), 1024(r)], value = emb[r0+r, 128*kc+p]
    embT = np.ascontiguousarray(emb.T)                     # [2048, 8192]
    emb8 = embT.reshape(16, 128, 8192).transpose(1, 0, 2)  # [128, 16, 8192]
    emb8 = emb8.astype(ml_dtypes.float8_e4m3)
    W8 = (W * WSCALE).reshape(16, 128, 2, 128).transpose(1, 2, 0, 3)
    W8 = np.ascontiguousarray(W8).astype(ml_dtypes.float8_e4m3)
    bS = np.ascontiguousarray((b * WSCALE).reshape(2, 128).T).astype(
        np.float32)
    ones_col = np.ones((128, 1), ml_dtypes.bfloat16)
    ones_row = np.ones((1, 512), ml_dtypes.bfloat16)
    ones8 = np.ones((128, 32), ml_dtypes.float8_e5m2)
    in_maps = []
    for c in range(8):
        sl = np.stack([emb8[:, :, 1024 * s:1024 * (s + 1)] for s in SLOTS[c]],
                      axis=1)                              # [128, 4, 16, 1024]
        in_maps.append({"emb8": np.ascontiguousarray(sl), "W8": W8, "bS": bS,
                        "ones_col": ones_col, "ones_row": ones_row,
                        "ones8": ones8})
    return in_maps


def _combine(results):
    # Staircase blocks (B0 diag, B4 pair): each core computes only
    # cols >= 256*pair; full sums are rowsum_P + colsum_P - diag-square
    # overlap (see kernel docstring).
    neg = np.zeros(8192, np.float64)
    pos = np.zeros(8192, np.float64)
    for c in range(8):
        S = SLOTS[c]
        rp = results[c]["rowpart"].astype(np.float64)
        rp = rp.reshape(128, 5, 8).transpose(1, 2, 0).reshape(5, 1024)
        cp = results[c]["colpart"].astype(np.float64)
        dq = results[c]["dsq"].astype(np.float64)
        dg = results[c]["diagexp"].astype(np.float64).ravel()
        sl = [np.s_[1024 * s:1024 * (s + 1)] for s in S]
        # B0 (diag, staircase): rows+cols-overlap, then self-sim removed
        neg[sl[0]] += rp[0] + cp[0] - dq[0] - dg
        neg[sl[0]] += rp[1]; neg[sl[1]] += cp[1]   # B1 (0,1)
        neg[sl[0]] += rp[2]; neg[sl[2]] += cp[2]   # B2 (0,2)
        neg[sl[1]] += rp[3]; neg[sl[3]] += cp[3]   # B3 (1,3)
        # B4 (0,3) staircase: this core's rows for slab S[0]; its cols
        # (minus overlap) credit slab S[3]; partner core supplies the rest
        neg[sl[0]] += rp[4]
        neg[sl[3]] += cp[4] - dq[1]
        if c < 4:
            ps = results[c]["possim"].astype(np.float64).ravel()
            pos[sl[0]] = ps
            pos[1024 * S[3]:1024 * (S[3] + 1)] = ps
    loss = -np.mean(10.0 * pos - np.log(neg))
    return np.float32(loss)


def run(embedded_data, W, b, trace=False):
    from concourse import bass_utils
    nc = _build()
    in_maps = _host_inputs(embedded_data, W, b)
    res = bass_utils.run_bass_kernel_spmd(nc, in_maps, core_ids=list(range(8)),
                                          trace=trace)
    return _combine(res.results), res


def kernel(embedded_data, W, b):
    loss, _ = run(embedded_data, W, b, trace=False)
    return np.asarray(loss, dtype=np.float32)


# revision 29
# speedup vs baseline: 1.0033x; 1.0033x over previous
"""NT-Xent contrastive loss on 8 Trainium2 NeuronCores (Bass/Tile), v10.

Slab-cover strategy (no collectives): core c loads fp8 slabs
S_c = {c, c+1, c+2, c+4} (mod 8); every slab pair meets on some core, so
each of the 36 unique 1024x1024 sim blocks is computed once globally.

Per-core kernel (vs the v1 baseline, 2x faster):
  * fp8(e4m3) inputs, host pre-laid-out to the exact SBUF tile layout
    [128, slab, kchunk, row] -> contiguous 16 KiB DMA descriptors
    (8 MiB/core vs 32 MiB); DMA issue order = HBM priority (W, slab-0
    chunks, then slabs 1-3).
  * All heavy matmuls run fp8 DoubleRow (K=256/instruction): head
    projection (W*32 fp8; bias added during PSUM evacuation via
    tensor_scalar), sim blocks, and exp column-sums over mb-paired
    fp8e5 exp tiles (ones[128,2,1] x exp[128,2,512]).
  * L2-normalize: normsq via ones-matmul on bf16 squares; rsqrt as
    Exp(-0.5*Ln(x)) with walrus steered (via get_activation_tables) to
    the single natural_log_exp_and_others table set -> no table thrash;
    1/norm broadcast by K=1 ones-row matmul; normalize-mult emits fp8.
  * Software pipeline: each block's 4 sim pairs interleave with the
    next slab's head chains (PE never FIFO-parks on the ACT exp
    backlog); colsum chains split so their last matmul never blocks;
    rsqrt chain emitted mid-phase.
  * Symmetric blocks (B0 diag, B4 = pair {c, c+4}, whose transpose is
    computed by core c+4) use a STAIRCASE: only cols >= 256*pair; host
    reassembles full sums as rowsum_P + colsum_P - diag-square overlap
    (dsq output). Cuts 6 of 40 exp-tile-equivalents of ScalarE work.
  * Diagonal self-sim exp(10*|u8_i|^2) recomputed exactly from the fp8
    vectors (DVE square + ones-matmul + ACT exp) and subtracted on host.
  * pos term: log(pos) = 10*possim computed without exp.
  * PSUM: head 2 banks, sim 4 (double-buffered), normsq 1, colsum 1.
"""
import numpy as np
import ml_dtypes

SLOTS = [(c, (c + 1) % 8, (c + 2) % 8, (c + 4) % 8) for c in range(8)]
WSCALE = 32.0  # power of two; normalize() cancels it exactly

_CACHE = {}


def _build():
    if "nc" in _CACHE:
        return _CACHE["nc"]
    import concourse.bacc as bacc
    import concourse.tile as tile
    import concourse.mybir as mybir

    F32 = mybir.dt.float32
    BF16 = mybir.dt.bfloat16
    F8E4 = mybir.dt.float8e4
    F8E5 = mybir.dt.float8e5
    AF = mybir.ActivationFunctionType
    ALU = mybir.AluOpType
    DR = mybir.MatmulPerfMode.DoubleRow

    # Steer walrus act-table selection: keep Exp/Ln only in the combined
    # natural_log_exp_and_others set so the kernel needs ONE table load
    # instead of thrashing exp_and_others <-> natural_log (1.28us each).
    _orig_gat = bacc.get_activation_tables

    def _gat(arch):
        t = _orig_gat(arch)
        for name, fns in t.items():
            if name != "natural_log_exp_and_others":
                fns.discard(mybir.ActivationFunctionType.Exp)
                fns.discard(mybir.ActivationFunctionType.Ln)
        return t

    bacc.get_activation_tables = _gat

    nc = bacc.Bacc("TRN2", num_devices=8, debug=False)
    a_emb = nc.dram_tensor("emb8", [128, 4, 16, 1024], F8E4,
                           kind="ExternalInput").ap()
    a_W = nc.dram_tensor("W8", [128, 16, 256], F8E4, kind="ExternalInput").ap()
    a_b = nc.dram_tensor("bS", [128, 2], F32, kind="ExternalInput").ap()
    a_oc = nc.dram_tensor("ones_col", [128, 1], BF16, kind="ExternalInput").ap()
    a_or = nc.dram_tensor("ones_row", [1, 512], BF16, kind="ExternalInput").ap()
    a_o8 = nc.dram_tensor("ones8", [128, 32], F8E5, kind="ExternalInput").ap()
    o_rp = nc.dram_tensor("rowpart", [128, 40], F32, kind="ExternalOutput").ap()
    o_cp = nc.dram_tensor("colpart", [5, 1024], F32, kind="ExternalOutput").ap()
    o_dg = nc.dram_tensor("diagexp", [1, 1024], F32, kind="ExternalOutput").ap()
    o_dq = nc.dram_tensor("dsq", [2, 1024], F32, kind="ExternalOutput").ap()
    o_ps = nc.dram_tensor("possim", [1, 1024], F32, kind="ExternalOutput").ap()

    with tile.TileContext(nc) as tc:
        with tc.tile_pool(name="sb", bufs=1) as sb, \
             tc.tile_pool(name="emb", bufs=4) as embp, \
             tc.tile_pool(name="hp", bufs=2) as hp, \
             tc.tile_pool(name="sq", bufs=2) as sqp, \
             tc.tile_pool(name="rn", bufs=2) as rnp, \
             tc.tile_pool(name="ln", bufs=2) as lnp, \
             tc.tile_pool(name="expp", bufs=9) as expp, \
             tc.tile_pool(name="headp", bufs=2, space="PSUM") as headp, \
             tc.tile_pool(name="simp", bufs=2, space="PSUM") as simp, \
             tc.tile_pool(name="nsp", bufs=1, space="PSUM") as nsp, \
             tc.tile_pool(name="csp", bufs=1, space="PSUM") as csp:

            # PE warm-up: HAM un-throttles the PE clock only after ~3.4us
            # of sustained matmul activity. Real data arrives ~12-14us in, so
            # burn dummy matmuls on a memset scratch tile from ~6us: the
            # first real matmul then runs at 2.4 GHz instead of 1.2.
            t_wu = sb.tile([128, 512], BF16, name="t_wu")
            nc.gpsimd.memset(t_wu[:], 0.0)
            p_wu = headp.tile([128, 512], F32, name="p_wu", tag="head")
            for _ in range(16):
                nc.tensor.matmul(p_wu[:], t_wu[:, 0:128], t_wu[:],
                                 start=True, stop=True)

            t_oc = sb.tile([128, 1], BF16, name="t_oc")
            nc.gpsimd.dma_start(t_oc[:], a_oc[:])
            t_or = sb.tile([1, 512], BF16, name="t_or")
            nc.gpsimd.dma_start(t_or[:], a_or[:])
            t_o8 = sb.tile([128, 2, 16], F8E5, name="t_o8")
            nc.gpsimd.dma_start(t_o8[:], a_o8.rearrange("p (a o) -> p a o", o=16))
            t_b = sb.tile([128, 2], F32, name="t_b")
            nc.gpsimd.dma_start(t_b[:], a_b[:])
            t_W = sb.tile([128, 16, 256], F8E4, name="t_W")
            nc.sync.dma_start(t_W[:], a_W[:])
            # slab 0 in four chunk tiles so head chains start on chunk 0
            t_e0c = []
            for ch in range(4):
                tec = embp.tile([128, 4, 1024], F8E4, name=f"t_e0c{ch}",
                                tag="emb")
                nc.sync.dma_start(tec[:], a_emb[:, 0, 4 * ch:4 * ch + 4, :])
                t_e0c.append(tec)
            t_e = [t_e0c]
            for k in range(1, 4):
                te = embp.tile([128, 16, 1024], F8E4, name=f"t_e{k}", tag="emb")
                nc.sync.dma_start(te[:], a_emb[:, k, :, :])
                t_e.append(te)

            # persistent normalized slabs (fp8) and staging accumulators
            t_on = [sb.tile([128, 2, 1024], F8E4, name=f"t_on{k}")
                    for k in range(4)]
            rp_st = sb.tile([128, 5, 8], F32, name="rp_st")
            cp_st = sb.tile([1, 5120], F32, name="cp_st")
            dq_st = sb.tile([1, 2048], F32, name="dq_st")
            dg_st = sb.tile([1, 1024], F32, name="dg_st")
            ps_st = sb.tile([1, 1024], F32, name="ps_st")

            def head_chain(k, th, tsq, h, dh):
                """one (h, dh) quarter of slab k's head projection."""
                ph = headp.tile([128, 512], F32, name="p_h", tag="head")
                for j in range(8):
                    if isinstance(t_e[k], list):
                        rhs = t_e[k][j // 2][:, 2 * (j % 2):2 * (j % 2) + 2,
                                             h * 512:(h + 1) * 512]
                    else:
                        rhs = t_e[k][:, 2 * j:2 * j + 2,
                                     h * 512:(h + 1) * 512]
                    nc.tensor.matmul(
                        ph[:],
                        t_W[:, 2 * j:2 * j + 2, dh * 128:(dh + 1) * 128],
                        rhs,
                        start=(j == 0), stop=(j == 7), perf_mode=DR)
                # evacuate + bias add (per-partition scalar b[d]) in one op
                nc.vector.tensor_scalar_add(
                    th[:, dh, h * 512:(h + 1) * 512], ph[:],
                    t_b[:, dh:dh + 1])
                nc.vector.tensor_tensor(
                    tsq[:, dh, h * 512:(h + 1) * 512],
                    th[:, dh, h * 512:(h + 1) * 512],
                    th[:, dh, h * 512:(h + 1) * 512], ALU.mult)

            def norm_half(tsq, rn, h):
                """normsq + rsqrt (Ln,Exp) for rows h*512..h*512+511."""
                pns = nsp.tile([1, 512], F32, name="p_ns", tag="ns")
                for dh in range(2):
                    nc.tensor.matmul(
                        pns[:], t_oc[:], tsq[:, dh, h * 512:(h + 1) * 512],
                        start=(dh == 0), stop=(dh == 1))
                tln = lnp.tile([1, 512], F32, name="t_ln", tag="ln")
                nc.scalar.activation(tln[:], pns[:], AF.Ln)
                nc.scalar.activation(rn[0:1, h * 512:(h + 1) * 512],
                                     tln[:], AF.Exp, scale=-0.5)

            def stage_finish(k, th, rn, h):
                """broadcast 1/norm, emit fp8 normalized half-slab h."""
                pbc = headp.tile([128, 512], F32, name="p_bc", tag="head")
                nc.tensor.matmul(pbc[:], t_or[0:1, 0:128],
                                 rn[0:1, h * 512:(h + 1) * 512],
                                 start=True, stop=True)
                for dh in range(2):
                    nc.vector.tensor_tensor(
                        t_on[k][:, dh, h * 512:(h + 1) * 512],
                        th[:, dh, h * 512:(h + 1) * 512],
                        pbc[:], ALU.mult)

            def sim_pair(bslot, a, bm, pair, stair=False):
                """two mb tiles of a sim block -> one fp8e5 exp pair tile.
                stair: compute only cols >= 256*pair (symmetric blocks whose
                transpose is covered elsewhere; host reassembles)."""
                lo = 256 * pair if stair else 0
                texp = expp.tile([128, 2, 1024], F8E5, name="t_exp", tag="exp")
                for half in range(2):
                    mb = 2 * pair + half
                    psim = simp.tile([128, 1024], F32, name="p_sim", tag="sim")
                    for nb in range(2):
                        nlo = max(nb * 512, lo)
                        if nlo < (nb + 1) * 512:
                            nc.tensor.matmul(
                                psim[:, nlo:(nb + 1) * 512],
                                t_on[a][:, :, mb * 128:(mb + 1) * 128],
                                t_on[bm][:, :, nlo:(nb + 1) * 512],
                                start=True, stop=True, perf_mode=DR)
                    nc.scalar.activation(
                        texp[:, half, lo:], psim[:, lo:], AF.Exp, scale=10.0,
                        accum_out=rp_st[:, bslot, mb:mb + 1])
                return texp

            def block_cs_start(bslot, texps, stair=False):
                """column-sum chains over all but the last contributing exp
                pair (whose exps may still be in flight on ACT)."""
                pcss = []
                for nb in range(2):
                    pairs = [p for p in range(4)
                             if not stair or 256 * p < (nb + 1) * 512]
                    pcs = csp.tile([1, 512], F32, name="p_cs", tag="cs")
                    pcss.append((pcs, pairs))
                    for pair in pairs[:-1]:
                        nlo = max(nb * 512, 256 * pair if stair else 0)
                        nc.tensor.matmul(
                            pcs[:, nlo - nb * 512:512], t_o8[:, :, 0:1],
                            texps[pair][:, :, nlo:(nb + 1) * 512],
                            start=(pair == pairs[0]), stop=False,
                            perf_mode=DR)
                return pcss

            def block_cs_finish(bslot, texps, pcss, stair=False):
                for nb in range(2):
                    pcs, pairs = pcss[nb]
                    pair = pairs[-1]
                    nlo = max(nb * 512, 256 * pair if stair else 0)
                    nc.tensor.matmul(
                        pcs[:, nlo - nb * 512:512], t_o8[:, :, 0:1],
                        texps[pair][:, :, nlo:(nb + 1) * 512],
                        start=False, stop=True, perf_mode=DR)
                    nc.vector.tensor_copy(
                        cp_st[0:1, 1024 * bslot + nb * 512:
                              1024 * bslot + (nb + 1) * 512],
                        pcs[:])
                # ship this block's outputs now (tiny DMAs, idle gpsimd queue)
                nc.gpsimd.dma_start(
                    o_cp[bslot:bslot + 1, :],
                    cp_st[0:1, 1024 * bslot:1024 * (bslot + 1)])
                nc.gpsimd.dma_start(o_rp[:, 8 * bslot:8 * (bslot + 1)],
                                    rp_st[:, bslot, :])

            def block_dsq(slot, texps):
                """column sums of the four diagonal 256x256 squares of a
                staircase block -> dq_st[slot*1024 : (slot+1)*1024]."""
                for pair in range(4):
                    pdq = csp.tile([1, 256], F32, name="p_dq", tag="cs")
                    nc.tensor.matmul(
                        pdq[:], t_o8[:, :, 0:1],
                        texps[pair][:, :, 256 * pair:256 * (pair + 1)],
                        start=True, stop=True, perf_mode=DR)
                    nc.vector.tensor_copy(
                        dq_st[0:1, 1024 * slot + 256 * pair:
                              1024 * slot + 256 * (pair + 1)], pdq[:])



            def phase(bslot, a, bm, nxt=None, cs_prev=None, stair=False):
                """block (bslot): 4 sim pairs, each followed by one head
                chain of the NEXT slab's stage, so the strict-FIFO PE queue
                always has head work while ACT drains the exp backlog and
                frees sim-psum buffers. Stage k+1's rsqrt chain is emitted
                mid-phase so its ACT ops sit ahead of half the exps."""
                th = tsq = rn = None
                pcss = None
                if cs_prev is not None:
                    pcss = block_cs_start(*cs_prev)
                if nxt is not None:
                    th = hp.tile([128, 2, 1024], BF16, name="t_h", tag="th")
                    tsq = sqp.tile([128, 2, 1024], BF16, name="t_sq", tag="sq")
                    rn = rnp.tile([1, 1024], BF16, name="t_rn", tag="rn")
                texps = []
                for pair in range(4):
                    texps.append(sim_pair(bslot, a, bm, pair, stair=stair))
                    if pair == 0 and pcss is not None:
                        block_cs_finish(cs_prev[0], cs_prev[1], pcss,
                                        stair=cs_prev[2])
                    if pair == 1 and pcss is not None and cs_prev[2]:
                        block_dsq(0 if cs_prev[0] == 0 else 1, cs_prev[1])
                    if nxt is not None:
                        head_chain(nxt, th, tsq, h=pair // 2, dh=pair % 2)
                        if pair == 1:
                            norm_half(tsq, rn, 0)
                        elif pair == 2:
                            stage_finish(nxt, th, rn, 0)
                        elif pair == 3:
                            norm_half(tsq, rn, 1)
                if nxt is not None:
                    stage_finish(nxt, th, rn, 1)
                return texps

            def colreduce_exp(src8, dst, scale):
                """dst[1,1024] = f(sum_d src8a[d,:]*src8b[d,:])."""
                tq = sqp.tile([128, 2, 1024], BF16, name="t_q", tag="sq")
                nc.vector.tensor_tensor(tq[:], src8[0][:], src8[1][:],
                                        ALU.mult)
                for nb in range(2):
                    pr = nsp.tile([1, 512], F32, name="p_r", tag="ns")
                    for dh in range(2):
                        nc.tensor.matmul(
                            pr[:], t_oc[:], tq[:, dh, nb * 512:(nb + 1) * 512],
                            start=(dh == 0), stop=(dh == 1))
                    if scale is None:
                        nc.vector.tensor_copy(
                            dst[0:1, nb * 512:(nb + 1) * 512], pr[:])
                    else:
                        nc.scalar.activation(
                            dst[0:1, nb * 512:(nb + 1) * 512], pr[:],
                            AF.Exp, scale=scale)

            # slab 0 head alone, then software-pipelined phases
            th0 = hp.tile([128, 2, 1024], BF16, name="t_h", tag="th")
            tsq0 = sqp.tile([128, 2, 1024], BF16, name="t_sq", tag="sq")
            rn0 = rnp.tile([1, 1024], BF16, name="t_rn", tag="rn")
            for h in range(2):
                for dh in range(2):
                    head_chain(0, th0, tsq0, h, dh)
                norm_half(tsq0, rn0, h)
                stage_finish(0, th0, rn0, h)

            tx0 = phase(0, 0, 0, nxt=1, stair=True)
            # diag exp values: exp(10 * |u8_i|^2) == exp(10 * sim_ii)
            colreduce_exp((t_on[0], t_on[0]), dg_st, 10.0)
            tx1 = phase(1, 0, 1, nxt=2, cs_prev=(0, tx0, True))
            tx2 = phase(2, 0, 2, nxt=3, cs_prev=(1, tx1, False))
            # pos: possim_i = sum_d u0[d,i]*u3[d,i]; host uses 10*possim
            colreduce_exp((t_on[0], t_on[3]), ps_st, None)
            tx3 = phase(3, 1, 3, cs_prev=(2, tx2, False))
            tx4 = phase(4, 0, 3, cs_prev=(3, tx3, False), stair=True)
            pc4 = block_cs_start(4, tx4, stair=True)
            block_cs_finish(4, tx4, pc4, stair=True)
            block_dsq(1, tx4)


            nc.gpsimd.dma_start(o_dg[:], dg_st[:])
            nc.gpsimd.dma_start(o_dq.rearrange("a r -> (a r)")[None, :],
                                dq_st[:])
            nc.gpsimd.dma_start(o_ps[:], ps_st[:])

    try:
        nc.compile()
    finally:
        bacc.get_activation_tables = _orig_gat
    _CACHE["nc"] = nc
    return nc


def _host_inputs(embedded_data, W, b):
    emb = np.asarray(embedded_data, dtype=np.float32)      # [8192, 2048]
    W = np.asarray(W, dtype=np.float32)
    b = np.asarray(b, dtype=np.float32)
    # slab s tile layout: [128(p), 16(kc), 1024(r)], value = emb[r0+r, 128*kc+p]
    embT = np.ascontiguousarray(emb.T)                     # [2048, 8192]
    emb8 = embT.reshape(16, 128, 8192).transpose(1, 0, 2)  # [128, 16, 8192]
    emb8 = emb8.astype(ml_dtypes.float8_e4m3)
    W8 = (W * WSCALE).reshape(16, 128, 256).transpose(1, 0, 2)
    W8 = np.ascontiguousarray(W8).astype(ml_dtypes.float8_e4m3)
    bS = np.ascontiguousarray((b * WSCALE).reshape(2, 128).T).astype(
        np.float32)
    ones_col = np.ones((128, 1), ml_dtypes.bfloat16)
    ones_row = np.ones((1, 512), ml_dtypes.bfloat16)
    ones8 = np.ones((128, 32), ml_dtypes.float8_e5m2)
    in_maps = []
    for c in range(8):
        sl = np.stack([emb8[:, :, 1024 * s:1024 * (s + 1)] for s in SLOTS[c]],
                      axis=1)                              # [128, 4, 16, 1024]
        in_maps.append({"emb8": np.ascontiguousarray(sl), "W8": W8, "bS": bS,
                        "ones_col": ones_col, "ones_row": ones_row,
                        "ones8": ones8})
    return in_maps


def _combine(results):
    # Staircase blocks (B0 diag, B4 pair): each core computes only
    # cols >= 256*pair; full sums are rowsum_P + colsum_P - diag-square
    # overlap (see kernel docstring).
    neg = np.zeros(8192, np.float64)
    pos = np.zeros(8192, np.float64)
    for c in range(8):
        S = SLOTS[c]
        rp = results[c]["rowpart"].astype(np.float64)
        rp = rp.reshape(128, 5, 8).transpose(1, 2, 0).reshape(5, 1024)
        cp = results[c]["colpart"].astype(np.float64)
        dq = results[c]["dsq"].astype(np.float64)
        dg = results[c]["diagexp"].astype(np.float64).ravel()
        sl = [np.s_[1024 * s:1024 * (s + 1)] for s in S]
        # B0 (diag, staircase): rows+cols-overlap, then self-sim removed
        neg[sl[0]] += rp[0] + cp[0] - dq[0] - dg
        neg[sl[0]] += rp[1]; neg[sl[1]] += cp[1]   # B1 (0,1)
        neg[sl[0]] += rp[2]; neg[sl[2]] += cp[2]   # B2 (0,2)
        neg[sl[1]] += rp[3]; neg[sl[3]] += cp[3]   # B3 (1,3)
        # B4 (0,3) staircase: this core's rows for slab S[0]; its cols
        # (minus overlap) credit slab S[3]; partner core supplies the rest
        neg[sl[0]] += rp[4]
        neg[sl[3]] += cp[4] - dq[1]
        if c < 4:
            ps = results[c]["possim"].astype(np.float64).ravel()
            pos[sl[0]] = ps
            pos[1024 * S[3]:1024 * (S[3] + 1)] = ps
    loss = -np.mean(10.0 * pos - np.log(neg))
    return np.float32(loss)


def run(embedded_data, W, b, trace=False):
    from concourse import bass_utils
    nc = _build()
    in_maps = _host_inputs(embedded_data, W, b)
    res = bass_utils.run_bass_kernel_spmd(nc, in_maps, core_ids=list(range(8)),
                                          trace=trace)
    return _combine(res.results), res


def kernel(embedded_data, W, b):
    loss, _ = run(embedded_data, W, b, trace=False)
    return np.asarray(loss, dtype=np.float32)
